# revision 71
# baseline (speedup 1.0000x reference)
"""Trainium2 Bass kernel for Longformer self-attention (B=2, S=4096, D=768, H=12, HD=64, W=256, G=32).

Sharding: 8 cores = 2 batches x 4 head-groups (3 heads each). Each core computes its
batch's projections restricted to its 192 output channels, runs banded + global
attention for its 3 heads, and returns an unnormalized transposed output
([3, 65, S]: rows 0-63 = head-dim, row 64 = softmax denominator z) plus the raw
global-query output [3, G, 65]; the host divides by z, transposes, and assembles.

Key design (final):
  - The 5 score-side projection chains (q01, k01, kg01, [q2|kg2], [k2|qg2]) run
    in fp8e4 with perf_mode=DoubleRow: K=256 per pass (3 passes over D=768
    instead of 6), ~1.7x faster. Weights are pre-scaled on the host
    (q-side x512, k-side x64; hidden_states x16); the dequant constant is
    fused into the PSUM->SBUF bias add as a tensor_scalar (mult, add).
    v/vg stay bf16 (fp8 there fails the accuracy budget; fp8 vvg is
    LDW-bound anyway).
  - All score matmuls run at K=128 full-array rate via ZERO-PADDED per-head
    query tiles (qZ[h] holds head h's 64 q-channels on the partition range of
    its k data, zeros elsewhere; the packed k tiles' cross-head terms are
    killed by the zeros). K=64 matmuls with changing weights cannot pipeline
    on the PE (measured fill+drain serialization, ~2x cost) -- avoided.
    (tile_position row/col packing measured: NO concurrency on this stack.)
  - global-query scores computed TRANSPOSED (kg block as the stationary
    operand vs the packed qgp tile), so the exp'd probs land in pb_gT
    directly -- replaces 32 PE transposes + vector copies.
  - hidden_states pre-transposed on host -> contiguous DMA loads; startup DMAs
    merged into few issues (each dma_start costs ~0.7us of engine queue) and
    sliced across the three DMA-issuing engines (sync/scalar/gpsimd); a
    1-element Exp warms the ACT table during startup.
  - band score blocks interleaved between projection/PV chains (same PE mode)
    so PSUM-slot waits on the softmax exp hide behind useful matmuls.
  - band mask applied as one strided bf16 multiply on the exp'd probs.
  - global-key PV contribution stays K=128 via zero-padded operands
    (exp_sg rows 96:128 = 0, per-head global-v at partitions 32h:32h+32).
  - v/vg bias via pre-broadcast bf16 add fused into the PSUM->SBUF copy.
Matmul inputs bf16 (PV/scores) and fp8e4 (projections), fp32 PSUM/softmax.
Measured: ~154us (best 152.6us) vs 177.9us baseline; rel err 1.607e-2.
"""
from collections import deque

import numpy as np
import ml_dtypes

import concourse.bass as bass
import concourse.mybir as mybir
import concourse.tile as tile
from concourse import bacc
from concourse.bass_utils import run_bass_kernel_spmd

B, S, D, H, HD = 2, 4096, 768, 12, 64
W = 256
G = 32
SCALE = 1.0 / np.float32(np.sqrt(HD))
KB = 128
NKB = S // KB     # 32
QSB = 512
NQSB = S // QSB   # 8
NKT = D // 128    # 6
NK2 = D // 256    # 3 fp8 DoubleRow passes
NNT = S // 512    # 8

BF = mybir.dt.bfloat16
F8 = mybir.dt.float8e4
F32 = mybir.dt.float32
AF = mybir.ActivationFunctionType
DR = mybir.MatmulPerfMode.DoubleRow
OP_MUL = mybir.AluOpType.mult
OP_ADD = mybir.AluOpType.add
bf16 = ml_dtypes.bfloat16
f8e4 = ml_dtypes.float8_e4m3

# fp8 scaling: weights q-side x512 / k-side x64, hidden_states x16.
XS = 16.0
WSQ = 512.0
WSK = 64.0
DQ_Q = 1.0 / (XS * WSQ)
DQ_K = 1.0 / (XS * WSK)

_cache = {}
PUMP_INTERLEAVE = True


def _span(kb):
    k0 = KB * kb
    qlo, qhi = max(0, k0 - 2 * KB), min(S, k0 + 3 * KB)
    return qlo, qhi, qlo - (k0 - 2 * KB), qhi - (k0 - 2 * KB)


def _build():
    nc = bacc.Bacc(None, target_bir_lowering=False)

    hsT_d = nc.declare_dram_parameter("hsT", [128, NNT, NKT, 512], BF, isOutput=False)
    hsT8_d = nc.declare_dram_parameter("hsT8", [128, NNT, NK2, 2, 512], F8,
                                       isOutput=False)
    w58_d = nc.declare_dram_parameter("w58", [128, 5, NK2, 2, 128], F8,
                                      isOutput=False)
    wqg_d = nc.declare_dram_parameter("wqg01", [128, NKT, 128], BF, isOutput=False)
    wvvg_d = nc.declare_dram_parameter("wvvg", [128, NKT, 384], BF, isOutput=False)
    bvvg_d = nc.declare_dram_parameter("bvvg", [1, 384], BF, isOutput=False)
    bias_d = nc.declare_dram_parameter("bias_t", [128, 8], F32, isOutput=False)
    masks_d = nc.declare_dram_parameter("masks", [128, 2, 128], BF, isOutput=False)
    out_d = nc.declare_dram_parameter("out", [3, 65, S], F32, isOutput=True)
    outg_d = nc.declare_dram_parameter("outg", [3, G, 65], F32, isOutput=True)

    with tile.TileContext(nc) as tc:
        with tc.tile_pool(name="persist", bufs=1) as pp:
            masks_t = pp.tile([128, 2, 128], BF)
            ones_t = pp.tile([1, 128], BF)

            # per-head zero-padded q; head h's live rows match its k tile rows
            qZ = [pp.tile([128, S], BF, name=f"qZ{i}") for i in range(3)]
            kT01 = pp.tile([128, S], BF)   # k: h0 rows 0:64, h1 rows 64:128
            kT2 = pp.tile([128, S], BF)    # k: h2 rows 0:64, rows 64:128 zero
            v_nat = pp.tile([128, NKB, 6, 65], BF)  # idx 0:3 = v, 3:6 = vg
            # exp_sg: rows 32h..32h+31 = head h's exp'd global-key scores;
            # rows 96:128 stay zero so K=128 matmuls vs vGp are exact.
            exp_sg = pp.tile([128, S], BF)

            with tc.tile_pool(name="ac", bufs=1) as ac:
                kgT01 = ac.tile([128, S], BF)
                kgT2 = ac.tile([128, S], BF)   # h2 rows 64:128, rows 0:64 zero
                # qgp: global queries packed [qg0 | qg1 | qg2] x 32 cols, each
                # live only on its head's channel rows (zeros elsewhere)
                qgp = ac.tile([128, 96], BF)
                vGp = ac.tile([128, 3, 65], BF)   # head h global-v at rows 32h:32h+32
                pb_gT = ac.tile([128, NKB, 96], BF)
                bvvg_b = ac.tile([128, 384], BF)  # bias broadcast over tokens
                og_acc = ac.tile([96, 65], F32)   # global-query PV accumulator

                with (
                    tc.tile_pool(name="aw", bufs=1) as aw,
                    tc.tile_pool(name="hst", bufs=3) as hstp,
                    tc.tile_pool(name="apsum", bufs=2, space="PSUM") as apsum,
                    tc.tile_pool(name="spsum", bufs=2, space="PSUM") as spsum,
                    tc.tile_pool(name="opsum", bufs=2, space="PSUM") as opsum,
                    tc.tile_pool(name="pbt", bufs=42) as pbtp,
                    tc.tile_pool(name="osb", bufs=2) as osbp,
                ):
                    w58_t = aw.tile([128, 5, NK2, 2, 128], F8)
                    wqg_t = aw.tile([128, NKT, 128], BF)
                    wvvg_t = aw.tile([128, NKT, 384], BF)
                    bvvg_t = aw.tile([1, 384], BF)
                    bias_t = aw.tile([128, 8], F32)
                    warm_t = aw.tile([1, 8], F32)
                    # issue startup DMAs sliced across the three DMA-issuing
                    # engines so the critical-path transfers run in parallel
                    hst0 = hstp.tile([128, NKT, 512], BF, tag="h16")
                    hst80 = hstp.tile([128, NK2, 2, 512], F8, tag="h8")
                    # gpsimd's first op feeds the scalar Exp-table warmup;
                    # startup DMAs are merged into few issues (each dma_start
                    # costs ~0.7us on the issuing engine's queue)
                    nc.gpsimd.memset(warm_t[:], 0.0)
                    nc.scalar.dma_start(bias_t[:], bias_d[:])
                    nc.scalar.activation(warm_t[0:1, 4:8], warm_t[0:1, 0:4], AF.Exp)
                    nc.sync.dma_start(w58_t[:, 0], w58_d[:, 0])
                    nc.gpsimd.dma_start(hst80[:], hsT8_d[:, 0])
                    nc.sync.dma_start(w58_t[:, 1:5], w58_d[:, 1:5])
                    nc.gpsimd.dma_start(hst0[:], hsT_d[:, 0])
                    # prefetch chunk nt=1 before the late-needed startup loads:
                    # by nt=1 the sync queue would otherwise still be draining
                    hst_1 = hstp.tile([128, NKT, 512], BF, tag="h16")
                    hst8_1 = hstp.tile([128, NK2, 2, 512], F8, tag="h8")
                    nc.sync.dma_start(hst8_1[:], hsT8_d[:, 1])
                    nc.sync.dma_start(hst_1[:], hsT_d[:, 1, :, :])
                    nc.sync.dma_start(masks_t[:], masks_d[:])
                    nc.scalar.dma_start(bvvg_t[:], bvvg_d[:])
                    nc.sync.dma_start(wvvg_t[:], wvvg_d[:])
                    nc.gpsimd.dma_start(wqg_t[:], wqg_d[:])
                    # zero-fill the dead halves of the padded tiles, chunked by
                    # 1024 columns and interleaved so the first consumers only
                    # wait on the first chunk
                    u32 = mybir.dt.uint32
                    nc.gpsimd.memset(ones_t[:], 1.0)
                    nc.gpsimd.memset(qgp[64:128, 0:32], 0.0)
                    nc.gpsimd.memset(qgp[0:64, 32:64], 0.0)
                    nc.gpsimd.memset(qgp[0:64, 64:96], 0.0)
                    nc.gpsimd.memset(vGp[:], 0.0)
                    for h in range(3):
                        nc.gpsimd.memset(vGp[32 * h:32 * h + 32, h, 64:65], 1.0)
                    for ch in range(0, S, 1024):
                        ce = ch + 1024
                        nc.gpsimd.memset(kgT2[0:64, ch:ce].bitcast(u32), 0)
                        nc.gpsimd.memset(qZ[0][64:128, ch:ce].bitcast(u32), 0)
                        nc.gpsimd.memset(qZ[1][0:64, ch:ce].bitcast(u32), 0)
                        nc.gpsimd.memset(qZ[2][64:128, ch:ce].bitcast(u32), 0)
                        nc.gpsimd.memset(kT2[64:128, ch:ce].bitcast(u32), 0)
                        nc.gpsimd.memset(exp_sg[96:128, ch:ce].bitcast(u32), 0)
                    nc.gpsimd.memset(v_nat[:, :, :, 64:65], 1.0)
                    nc.gpsimd.memset(og_acc[:], 0.0)

                    pbt = {}
                    band_q = deque()
                    state = {"kb_done": 0, "qs_done": 0, "kb_next": 0}

                    def ktile(h):
                        return kT01 if h < 2 else kT2

                    def mm_score(t, h, kb, a, b2):
                        k0 = KB * kb
                        qlo, qhi, llo, lhi = _span(kb)
                        nc.tensor.matmul(
                            t[:, a:b2],
                            ktile(h)[:, k0:k0 + KB],
                            qZ[h][:, qlo + (a - llo):qlo + (a - llo) + (b2 - a)])

                    def exp_mask(ps, kb, h):
                        qlo, qhi, llo, lhi = _span(kb)
                        t_ = pbtp.tile([128, 640], BF, tag="pb")
                        nc.scalar.activation(t_[:, llo:lhi], ps[:, llo:lhi], AF.Exp)
                        tv = t_.rearrange("p (o j) -> p o j", o=5)
                        if llo == 0 and lhi == 640:
                            nc.vector.tensor_mul(tv[:, 0:5:4, :], tv[:, 0:5:4, :],
                                                 masks_t[:])
                        elif llo == 0:
                            nc.vector.tensor_mul(tv[:, 0, :], tv[:, 0, :],
                                                 masks_t[:, 0, :])
                        else:
                            nc.vector.tensor_mul(tv[:, 4, :], tv[:, 4, :],
                                                 masks_t[:, 1, :])
                        pbt[(kb, h)] = t_

                    def queue_kb(kb):
                        qlo, qhi, llo, lhi = _span(kb)
                        pieces = [(a, b) for (a, b) in
                                  [(llo, min(lhi, 512)), (max(llo, 512), lhi)] if a < b]

                        def part1():
                            t0 = spsum.tile([128, 1024], F32, tag="sc", name=f"s{kb}_0")
                            t1 = spsum.tile([128, 1024], F32, tag="sc", name=f"s{kb}_1")
                            for (a, b2) in pieces:
                                mm_score(t0, 0, kb, a, b2)
                                mm_score(t1, 1, kb, a, b2)
                            exp_mask(t0, kb, 0)
                            exp_mask(t1, kb, 1)

                        def part2():
                            t2 = spsum.tile([128, 1024], F32, tag="sc", name=f"s{kb}_2")
                            for (a, b2) in pieces:
                                mm_score(t2, 2, kb, a, b2)
                            exp_mask(t2, kb, 2)
                            state["kb_done"] += 1

                        band_q.append(part1)
                        band_q.append(part2)

                    def pump(n=1):
                        if PUMP_INTERLEAVE:
                            for _ in range(n):
                                if band_q:
                                    band_q.popleft()()

                    def pump_all():
                        while band_q:
                            band_q.popleft()()

                    def do_pv(qs):
                        q0 = QSB * qs
                        kbs = list(range(max(0, 4 * qs - 2), min(NKB, 4 * qs + 6)))
                        for h in range(3):
                            po = opsum.tile([96, 512], F32, tag="po")
                            nc.tensor.matmul(po[0:65, :], vGp[:, h, :],
                                             exp_sg[:, q0:q0 + 512],
                                             start=True, stop=False)
                            for i, kb in enumerate(kbs):
                                k0 = KB * kb
                                qlo, qhi, llo, lhi = _span(kb)
                                a, b2 = max(qlo, q0), min(qhi, q0 + QSB)
                                la = a - (k0 - 2 * KB)
                                nc.tensor.matmul(po[0:65, a - q0:b2 - q0],
                                                 v_nat[:, kb, h, :],
                                                 pbt[(kb, h)][:, la:la + (b2 - a)],
                                                 start=False, stop=(i == len(kbs) - 1))
                            ob = osbp.tile([65, 512], F32, tag="ob")
                            if h == 1:
                                nc.scalar.copy(ob[:], po[0:65, :])
                                nc.scalar.dma_start(out_d[h, :, q0:q0 + 512], ob[:])
                            else:
                                nc.vector.tensor_copy(ob[:], po[0:65, :])
                                if h == 0:
                                    nc.sync.dma_start(out_d[h, :, q0:q0 + 512], ob[:])
                                else:
                                    nc.gpsimd.dma_start(out_d[h, :, q0:q0 + 512],
                                                        ob[:])
                            pump()

                    def pump_pv(nt):
                        while state["qs_done"] < NQSB \
                                and 4 * state["qs_done"] + 5 <= state["kb_done"] - 1 \
                                and state["qs_done"] <= nt - 1:
                            do_pv(state["qs_done"])
                            state["qs_done"] += 1

                    def do_go(blks):
                        # one 195-col MM per block: out[g, (h,c)] includes the
                        # wanted diagonal blocks og[32h:32h+32, 65h:65h+65];
                        # off-diagonal products are computed but ignored.
                        # Beats 3 head-MMs of 65 cols (issue-overhead-bound).
                        gp = apsum.tile([96, 3, 65], F32, tag="pp",
                                        name=f"go{blks[0]}")
                        for i, t in enumerate(blks):
                            nc.tensor.matmul(gp[:], pb_gT[:, t, :],
                                             v_nat[:, t, 3:6, :],
                                             start=(i == 0),
                                             stop=(i == len(blks) - 1))
                        for h in range(3):
                            nc.vector.tensor_copy(og_acc[32 * h:32 * h + 32, :],
                                                  gp[32 * h:32 * h + 32, h, :])

                    nxt = {0: (hst0, hst80), 1: (hst_1, hst8_1)}
                    for nt in range(NNT):
                        c0 = 512 * nt
                        hst, hst8 = nxt.pop(nt)
                        # queue band blocks whose q/k spans are fully written
                        # (chunks <= nt-1); the rest after this chunk's m-chains
                        while state["kb_next"] <= min(4 * nt - 3, NKB - 1):
                            queue_kb(state["kb_next"])
                            state["kb_next"] += 1
                        # band work fills the PE while this chunk's hst8 DMA
                        # is still in flight
                        pump(2)
                        for c in range(5):
                            ps = apsum.tile([128, 512], F32, tag="pp")
                            for k2 in range(NK2):
                                nc.tensor.matmul(ps[:], w58_t[:, c, k2],
                                                 hst8[:, k2],
                                                 start=(k2 == 0), stop=(k2 == NK2 - 1),
                                                 perf_mode=DR)
                            if c == 0:
                                nc.vector.tensor_scalar(
                                    qZ[0][0:64, c0:c0 + 512], ps[0:64, :],
                                    DQ_Q, bias_t[0:64, 0:1], op0=OP_MUL, op1=OP_ADD)
                                nc.vector.tensor_scalar(
                                    qZ[1][64:128, c0:c0 + 512], ps[64:128, :],
                                    DQ_Q, bias_t[64:128, 0:1], op0=OP_MUL, op1=OP_ADD)
                            elif c == 1:
                                nc.vector.tensor_scalar(
                                    kT01[:, c0:c0 + 512], ps[:],
                                    DQ_K, bias_t[:, 1:2], op0=OP_MUL, op1=OP_ADD)
                            elif c == 2:
                                nc.vector.tensor_scalar(
                                    kgT01[:, c0:c0 + 512], ps[:],
                                    DQ_K, bias_t[:, 2:3], op0=OP_MUL, op1=OP_ADD)
                            elif c == 3:
                                nc.vector.tensor_scalar(
                                    qZ[2][0:64, c0:c0 + 512], ps[0:64, :],
                                    DQ_Q, bias_t[0:64, 3:4], op0=OP_MUL, op1=OP_ADD)
                                nc.vector.tensor_scalar(
                                    kgT2[64:128, c0:c0 + 512], ps[64:128, :],
                                    DQ_K, bias_t[64:128, 3:4], op0=OP_MUL, op1=OP_ADD)
                            else:
                                nc.vector.tensor_scalar(
                                    kT2[0:64, c0:c0 + 512], ps[0:64, :],
                                    DQ_K, bias_t[0:64, 4:5], op0=OP_MUL, op1=OP_ADD)
                                if nt == 0:
                                    nc.vector.tensor_scalar(
                                        qgp[64:128, 64:96], ps[64:128, 0:G],
                                        DQ_Q, bias_t[64:128, 4:5],
                                        op0=OP_MUL, op1=OP_ADD)
                            pump()
                        while state["kb_next"] <= min(4 * nt + 1, NKB - 1):
                            queue_kb(state["kb_next"])
                            state["kb_next"] += 1
                        if nt == 0:
                            psq = apsum.tile([128, 512], F32, tag="pp")
                            for kt in range(NKT):
                                nc.tensor.matmul(psq[:, 0:G], wqg_t[:, kt, :],
                                                 hst[:, kt, 0:G],
                                                 start=(kt == 0), stop=(kt == NKT - 1))
                            nc.vector.tensor_scalar_add(qgp[0:64, 0:32], psq[0:64, 0:G],
                                                        bias_t[0:64, 5:6])
                            nc.vector.tensor_scalar_add(qgp[64:128, 32:64],
                                                        psq[64:128, 0:G],
                                                        bias_t[64:128, 5:6])
                            # broadcast bvvg to all partitions via ones-matmul
                            psb = apsum.tile([128, 512], F32, tag="pp")
                            nc.tensor.matmul(psb[:, 0:384], ones_t[:, 0:128], bvvg_t[:],
                                             start=True, stop=True)
                            nc.vector.tensor_copy(bvvg_b[:], psb[:, 0:384])
                        for s4 in range(4):
                            sb = 4 * nt + s4
                            psv = apsum.tile([128, 512], F32, tag="pp")
                            for kt in range(NKT):
                                nc.tensor.matmul(psv[:, 0:384],
                                                 hst[:, kt, 128 * s4:128 * s4 + 128],
                                                 wvvg_t[:, kt, :],
                                                 start=(kt == 0), stop=(kt == NKT - 1))
                            nc.vector.tensor_add(
                                v_nat[:, sb, :, 0:64],
                                psv[:, 0:384].rearrange("p (h e) -> p h e", h=6),
                                bvvg_b[:, 0:384].rearrange("p (h e) -> p h e", h=6))
                            pump()
                        if nt == 0:
                            # per-head global v/ones at partitions 32h:32h+32
                            nc.vector.tensor_copy(vGp[0:32, 0, 0:64],
                                                  v_nat[0:32, 0, 0, 0:64])
                            nc.sync.dma_start(vGp[32:64, 1, 0:64],
                                              v_nat[0:32, 0, 1, 0:64])
                            nc.sync.dma_start(vGp[64:96, 2, 0:64],
                                              v_nat[0:32, 0, 2, 0:64])
                        pump_pv(nt)
                        pump(2)
                        # ---- (128,32)-col-tiled scores: sg + global-query ----
                        pssg = opsum.tile([96, 512], F32, tag="po", name=f"sg{nt}")
                        for h in range(3):
                            nc.tensor.matmul(pssg[32 * h:32 * h + 32, :],
                                             ktile(h)[:, 0:G],
                                             qZ[h][:, c0:c0 + 512])
                        nc.scalar.activation(exp_sg[0:96, c0:c0 + 512], pssg[:], AF.Exp)
                        # global-query scores computed TRANSPOSED (tokens on
                        # partitions): kg block is the stationary operand, so the
                        # exp'd probs land in pb_gT directly -- no PE transpose
                        gcb = apsum.tile([128, 4, 96], F32, tag="pp", name=f"gc{nt}")
                        for j in range(4):
                            tb = 128 * (4 * nt + j)
                            nc.tensor.matmul(gcb[:, j, 0:64],
                                             kgT01[:, tb:tb + 128], qgp[:, 0:64])
                            nc.tensor.matmul(gcb[:, j, 64:96],
                                             kgT2[:, tb:tb + 128], qgp[:, 64:96])
                        nc.scalar.activation(pb_gT[:, 4 * nt:4 * nt + 4, :], gcb[:],
                                             AF.Exp)
                        pump(2)
                        if nt + 2 < NNT:
                            # prefetch chunk nt+2's inputs so the next-next
                            # iteration's chains never wait on the DMA
                            h16 = hstp.tile([128, NKT, 512], BF, tag="h16")
                            h8 = hstp.tile([128, NK2, 2, 512], F8, tag="h8")
                            nc.sync.dma_start(h8[:], hsT8_d[:, nt + 2])
                            nc.sync.dma_start(h16[:], hsT_d[:, nt + 2, :, :])
                            nxt[nt + 2] = (h16, h8)

                    # tail: drain band queue, pv(6); the global-query PV chain
                    # runs before pv(7) so its copies/DMA overlap pv(7)'s matmuls
                    pump_all()
                    while state["qs_done"] <= 6:
                        do_pv(state["qs_done"])
                        state["qs_done"] += 1
                    while state["kb_next"] < NKB:
                        queue_kb(state["kb_next"])
                        state["kb_next"] += 1
                    pump_all()
                    do_go(list(range(NKB)))
                    do_pv(7)
                    nc.sync.dma_start(outg_d[0], og_acc[0:32, :])
                    nc.scalar.dma_start(outg_d[1], og_acc[32:64, :])
                    nc.gpsimd.dma_start(outg_d[2], og_acc[64:96, :])

    nc.compile()
    return nc


def _prep_inputs(inputs):
    hs = np.asarray(inputs["hidden_states"], dtype=np.float32)
    j = np.arange(KB)[None, :]
    p = np.arange(KB)[:, None]
    masks = np.stack([(j >= p), (j <= p)], axis=1).astype(bf16)  # [128, 2, 128]

    def wtiles(w):
        n = w.shape[1]
        return np.ascontiguousarray(w.reshape(NKT, 128, n).transpose(1, 0, 2)).astype(bf16)

    def fp8(x):
        return np.clip(x, -240.0, 240.0).astype(f8e4)

    maps = []
    for c in range(8):
        b, hg = c // 4, c % 4
        cols = slice(192 * hg, 192 * hg + 192)
        Wq = np.asarray(inputs["Wq"], np.float32)[:, cols] * SCALE
        bq = np.asarray(inputs["bq"], np.float32)[cols] * SCALE
        Wqg = np.asarray(inputs["Wqg"], np.float32)[:, cols] * SCALE
        bqg = np.asarray(inputs["bqg"], np.float32)[cols] * SCALE
        Wk = np.asarray(inputs["Wk"], np.float32)[:, cols]
        bk = np.asarray(inputs["bk"], np.float32)[cols]
        Wkg = np.asarray(inputs["Wkg"], np.float32)[:, cols]
        bkg = np.asarray(inputs["bkg"], np.float32)[cols]
        Wv = np.asarray(inputs["Wv"], np.float32)[:, cols]
        bv = np.asarray(inputs["bv"], np.float32)[cols]
        Wvg = np.asarray(inputs["Wvg"], np.float32)[:, cols]
        bvg = np.asarray(inputs["bvg"], np.float32)[cols]

        # 5 chains: q01, k01, kg01, [q2|kg2], [k2|qg2]; fp8-prescaled
        w5 = np.concatenate([
            Wq[:, 0:128] * WSQ,
            Wk[:, 0:128] * WSK,
            Wkg[:, 0:128] * WSK,
            np.concatenate([Wq[:, 128:192] * WSQ, Wkg[:, 128:192] * WSK], axis=1),
            np.concatenate([Wk[:, 128:192] * WSK, Wqg[:, 128:192] * WSQ], axis=1),
        ], axis=1)  # [768, 640]
        # channel c -> (k2 = c//256, i = (c//128)%2, p = c%128)
        w58 = np.ascontiguousarray(
            fp8(w5).reshape(NK2, 2, 128, 5, 128).transpose(2, 3, 0, 1, 4))

        bias_t = np.zeros((128, 8), np.float32)
        bias_t[:, 0] = bq[0:128]
        bias_t[:, 1] = bk[0:128]
        bias_t[:, 2] = bkg[0:128]
        bias_t[0:64, 3], bias_t[64:128, 3] = bq[128:192], bkg[128:192]
        bias_t[0:64, 4], bias_t[64:128, 4] = bk[128:192], bqg[128:192]
        bias_t[0:64, 5], bias_t[64:128, 5] = bqg[0:64], bqg[64:128]

        hsT = np.ascontiguousarray(
            hs[b].T.reshape(NKT, 128, NNT, 512).transpose(1, 2, 0, 3)).astype(bf16)
        hsT8 = np.ascontiguousarray(
            fp8(hs[b].T * XS).reshape(NK2, 2, 128, NNT, 512).transpose(2, 3, 0, 1, 4))

        maps.append({
            "hsT": hsT,
            "hsT8": hsT8,
            "w58": w58,
            "wqg01": wtiles(Wqg[:, 0:128]),
            "wvvg": wtiles(np.concatenate([Wv, Wvg], axis=1)),
            "bvvg": np.concatenate([bv, bvg])[None, :].astype(bf16),
            "bias_t": bias_t,
            "masks": masks,
        })
    return maps


def kernel(**inputs):
    g = int(np.asarray(inputs["num_global"]))
    assert g == G, f"kernel compiled for num_global=32, got {g}"
    if "nc" not in _cache:
        _cache["nc"] = _build()
    nc = _cache["nc"]
    in_maps = _prep_inputs(inputs)
    res = run_bass_kernel_spmd(nc, in_maps, list(range(8)))
    return assemble(res.results)


def assemble(results):
    out = np.zeros((B, S, D), np.float32)
    for c in range(8):
        b, hg = c // 4, c % 4
        o = results[c]["out"]          # [3, 65, S]
        og = results[c]["outg"]        # [3, G, 65]
        for h in range(3):
            col = 192 * hg + 64 * h
            out[b, :, col:col + 64] = (o[h, 0:64] / o[h, 64]).T
            out[b, 0:G, col:col + 64] = og[h, :, 0:64] / og[h, :, 64:65]
    return out


# revision 72
# speedup vs baseline: 1.0068x; 1.0068x over previous
"""Trainium2 Bass kernel for Longformer self-attention (B=2, S=4096, D=768, H=12, HD=64, W=256, G=32).

Sharding: 8 cores = 2 batches x 4 head-groups (3 heads each). Each core computes its
batch's projections restricted to its 192 output channels, runs banded + global
attention for its 3 heads, and returns an unnormalized transposed output
([3, 65, S]: rows 0-63 = head-dim, row 64 = softmax denominator z) plus the raw
global-query output [3, G, 65]; the host divides by z, transposes, and assembles.

Key design (final):
  - The 5 score-side projection chains (q01, k01, kg01, [q2|kg2], [k2|qg2]) run
    in fp8e4 with perf_mode=DoubleRow: K=256 per pass (3 passes over D=768
    instead of 6), ~1.7x faster. Weights are pre-scaled on the host
    (q-side x512, k-side x64; hidden_states x16); the dequant constant is
    fused into the PSUM->SBUF bias add as a tensor_scalar (mult, add).
    v/vg stay bf16 (fp8 there fails the accuracy budget; fp8 vvg is
    LDW-bound anyway).
  - All score matmuls run at K=128 full-array rate via ZERO-PADDED per-head
    query tiles (qZ[h] holds head h's 64 q-channels on the partition range of
    its k data, zeros elsewhere; the packed k tiles' cross-head terms are
    killed by the zeros). K=64 matmuls with changing weights cannot pipeline
    on the PE (measured fill+drain serialization, ~2x cost) -- avoided.
    (tile_position row/col packing measured: NO concurrency on this stack.)
  - global-query scores computed TRANSPOSED (kg block as the stationary
    operand vs the packed qgp tile), so the exp'd probs land in pb_gT
    directly -- replaces 32 PE transposes + vector copies.
  - hidden_states pre-transposed on host -> contiguous DMA loads; startup DMAs
    merged into few issues (each dma_start costs ~0.7us of engine queue) and
    sliced across the three DMA-issuing engines (sync/scalar/gpsimd); a
    1-element Exp warms the ACT table during startup.
  - band score blocks interleaved between projection/PV chains (same PE mode)
    so PSUM-slot waits on the softmax exp hide behind useful matmuls.
  - band mask applied as one strided bf16 multiply on the exp'd probs.
  - global-key PV contribution stays K=128 via zero-padded operands
    (exp_sg rows 96:128 = 0, per-head global-v at partitions 32h:32h+32).
  - v/vg bias via pre-broadcast bf16 add fused into the PSUM->SBUF copy.
Matmul inputs bf16 (PV/scores) and fp8e4 (projections), fp32 PSUM/softmax.
Measured: ~154us (best 152.6us) vs 177.9us baseline; rel err 1.607e-2.
"""
from collections import deque

import numpy as np
import ml_dtypes

import concourse.bass as bass
import concourse.mybir as mybir
import concourse.tile as tile
from concourse import bacc
from concourse.bass_utils import run_bass_kernel_spmd

B, S, D, H, HD = 2, 4096, 768, 12, 64
W = 256
G = 32
SCALE = 1.0 / np.float32(np.sqrt(HD))
KB = 128
NKB = S // KB     # 32
QSB = 512
NQSB = S // QSB   # 8
NKT = D // 128    # 6
NK2 = D // 256    # 3 fp8 DoubleRow passes
NNT = S // 512    # 8

BF = mybir.dt.bfloat16
F8 = mybir.dt.float8e4
F32 = mybir.dt.float32
AF = mybir.ActivationFunctionType
DR = mybir.MatmulPerfMode.DoubleRow
OP_MUL = mybir.AluOpType.mult
OP_ADD = mybir.AluOpType.add
bf16 = ml_dtypes.bfloat16
f8e4 = ml_dtypes.float8_e4m3

# fp8 scaling: weights q-side x512 / k-side x64, hidden_states x16.
XS = 16.0
WSQ = 512.0
WSK = 64.0
DQ_Q = 1.0 / (XS * WSQ)
DQ_K = 1.0 / (XS * WSK)

_cache = {}
PUMP_INTERLEAVE = True


def _span(kb):
    k0 = KB * kb
    qlo, qhi = max(0, k0 - 2 * KB), min(S, k0 + 3 * KB)
    return qlo, qhi, qlo - (k0 - 2 * KB), qhi - (k0 - 2 * KB)


def _build():
    nc = bacc.Bacc(None, target_bir_lowering=False)

    hsT_d = nc.declare_dram_parameter("hsT", [128, NNT, NKT, 512], BF, isOutput=False)
    hsT8_d = nc.declare_dram_parameter("hsT8", [128, NNT, NK2, 2, 512], F8,
                                       isOutput=False)
    w58_d = nc.declare_dram_parameter("w58", [128, 5, NK2, 2, 128], F8,
                                      isOutput=False)
    wqg_d = nc.declare_dram_parameter("wqg01", [128, NKT, 128], BF, isOutput=False)
    wvvg_d = nc.declare_dram_parameter("wvvg", [128, NKT, 384], BF, isOutput=False)
    bvvg_d = nc.declare_dram_parameter("bvvg", [1, 384], BF, isOutput=False)
    bias_d = nc.declare_dram_parameter("bias_t", [128, 8], F32, isOutput=False)
    masks_d = nc.declare_dram_parameter("masks", [128, 2, 128], BF, isOutput=False)
    out_d = nc.declare_dram_parameter("out", [3, 65, S], F32, isOutput=True)
    outg_d = nc.declare_dram_parameter("outg", [3, G, 65], F32, isOutput=True)

    with tile.TileContext(nc) as tc:
        with tc.tile_pool(name="persist", bufs=1) as pp:
            masks_t = pp.tile([128, 2, 128], BF)
            ones_t = pp.tile([1, 128], BF)

            # per-head zero-padded q; head h's live rows match its k tile rows
            qZ = [pp.tile([128, S], BF, name=f"qZ{i}") for i in range(3)]
            kT01 = pp.tile([128, S], BF)   # k: h0 rows 0:64, h1 rows 64:128
            kT2 = pp.tile([128, S], BF)    # k: h2 rows 0:64, rows 64:128 zero
            v_nat = pp.tile([128, NKB, 6, 65], BF)  # idx 0:3 = v, 3:6 = vg
            # exp_sg: rows 32h..32h+31 = head h's exp'd global-key scores;
            # rows 96:128 stay zero so K=128 matmuls vs vGp are exact.
            exp_sg = pp.tile([128, S], BF)

            with tc.tile_pool(name="ac", bufs=1) as ac:
                kgT01 = ac.tile([128, S], BF)
                kgT2 = ac.tile([128, S], BF)   # h2 rows 64:128, rows 0:64 zero
                # qgp: global queries packed [qg0 | qg1 | qg2] x 32 cols, each
                # live only on its head's channel rows (zeros elsewhere)
                qgp = ac.tile([128, 96], BF)
                vGp = ac.tile([128, 3, 65], BF)   # head h global-v at rows 32h:32h+32
                pb_gT = ac.tile([128, NKB, 96], BF)
                bvvg_b = ac.tile([128, 384], BF)  # bias broadcast over tokens
                og_acc = ac.tile([96, 65], F32)   # global-query PV accumulator

                with (
                    tc.tile_pool(name="aw", bufs=1) as aw,
                    tc.tile_pool(name="hst", bufs=3) as hstp,
                    tc.tile_pool(name="apsum", bufs=2, space="PSUM") as apsum,
                    tc.tile_pool(name="spsum", bufs=2, space="PSUM") as spsum,
                    tc.tile_pool(name="opsum", bufs=2, space="PSUM") as opsum,
                    tc.tile_pool(name="pbt", bufs=42) as pbtp,
                    tc.tile_pool(name="osb", bufs=2) as osbp,
                ):
                    w58_t = aw.tile([128, 5, NK2, 2, 128], F8)
                    wqg_t = aw.tile([128, NKT, 128], BF)
                    wvvg_t = aw.tile([128, NKT, 384], BF)
                    bvvg_t = aw.tile([1, 384], BF)
                    bias_t = aw.tile([128, 8], F32)
                    warm_t = aw.tile([1, 8], F32)
                    # issue startup DMAs sliced across the three DMA-issuing
                    # engines so the critical-path transfers run in parallel
                    hst0 = hstp.tile([128, NKT, 512], BF, tag="h16")
                    hst80 = hstp.tile([128, NK2, 2, 512], F8, tag="h8")
                    # gpsimd's first op feeds the scalar Exp-table warmup;
                    # startup DMAs are merged into few issues (each dma_start
                    # costs ~0.7us on the issuing engine's queue)
                    nc.gpsimd.memset(warm_t[:], 0.0)
                    nc.scalar.activation(warm_t[0:1, 4:8], warm_t[0:1, 0:4], AF.Exp)
                    nc.scalar.dma_start(bias_t[:], bias_d[:])
                    nc.sync.dma_start(w58_t[:, 0], w58_d[:, 0])
                    nc.gpsimd.dma_start(hst80[:], hsT8_d[:, 0])
                    nc.sync.dma_start(w58_t[:, 1:5], w58_d[:, 1:5])
                    nc.gpsimd.dma_start(hst0[:], hsT_d[:, 0])
                    # prefetch chunk nt=1 before the late-needed startup loads:
                    # by nt=1 the sync queue would otherwise still be draining
                    hst_1 = hstp.tile([128, NKT, 512], BF, tag="h16")
                    hst8_1 = hstp.tile([128, NK2, 2, 512], F8, tag="h8")
                    nc.sync.dma_start(hst8_1[:], hsT8_d[:, 1])
                    nc.sync.dma_start(hst_1[:], hsT_d[:, 1, :, :])
                    nc.sync.dma_start(masks_t[:], masks_d[:])
                    nc.scalar.dma_start(bvvg_t[:], bvvg_d[:])
                    nc.sync.dma_start(wvvg_t[:], wvvg_d[:])
                    nc.gpsimd.dma_start(wqg_t[:], wqg_d[:])
                    # zero-fill the dead halves of the padded tiles, chunked by
                    # 1024 columns and interleaved so the first consumers only
                    # wait on the first chunk
                    u32 = mybir.dt.uint32
                    nc.gpsimd.memset(ones_t[:], 1.0)
                    nc.gpsimd.memset(qgp[64:128, 0:32], 0.0)
                    nc.gpsimd.memset(qgp[0:64, 32:64], 0.0)
                    nc.gpsimd.memset(qgp[0:64, 64:96], 0.0)
                    nc.gpsimd.memset(vGp[:], 0.0)
                    for h in range(3):
                        nc.gpsimd.memset(vGp[32 * h:32 * h + 32, h, 64:65], 1.0)
                    for ch in range(0, S, 1024):
                        ce = ch + 1024
                        nc.gpsimd.memset(kgT2[0:64, ch:ce].bitcast(u32), 0)
                        nc.gpsimd.memset(qZ[0][64:128, ch:ce].bitcast(u32), 0)
                        nc.gpsimd.memset(qZ[1][0:64, ch:ce].bitcast(u32), 0)
                        nc.gpsimd.memset(qZ[2][64:128, ch:ce].bitcast(u32), 0)
                        nc.gpsimd.memset(kT2[64:128, ch:ce].bitcast(u32), 0)
                        nc.gpsimd.memset(exp_sg[96:128, ch:ce].bitcast(u32), 0)
                    nc.gpsimd.memset(v_nat[:, :, :, 64:65], 1.0)
                    nc.gpsimd.memset(og_acc[:], 0.0)

                    pbt = {}
                    band_q = deque()
                    state = {"kb_done": 0, "qs_done": 0, "kb_next": 0}

                    def ktile(h):
                        return kT01 if h < 2 else kT2

                    def mm_score(t, h, kb, a, b2):
                        k0 = KB * kb
                        qlo, qhi, llo, lhi = _span(kb)
                        nc.tensor.matmul(
                            t[:, a:b2],
                            ktile(h)[:, k0:k0 + KB],
                            qZ[h][:, qlo + (a - llo):qlo + (a - llo) + (b2 - a)])

                    def exp_mask(ps, kb, h):
                        qlo, qhi, llo, lhi = _span(kb)
                        t_ = pbtp.tile([128, 640], BF, tag="pb")
                        nc.scalar.activation(t_[:, llo:lhi], ps[:, llo:lhi], AF.Exp)
                        tv = t_.rearrange("p (o j) -> p o j", o=5)
                        if llo == 0 and lhi == 640:
                            nc.vector.tensor_mul(tv[:, 0:5:4, :], tv[:, 0:5:4, :],
                                                 masks_t[:])
                        elif llo == 0:
                            nc.vector.tensor_mul(tv[:, 0, :], tv[:, 0, :],
                                                 masks_t[:, 0, :])
                        else:
                            nc.vector.tensor_mul(tv[:, 4, :], tv[:, 4, :],
                                                 masks_t[:, 1, :])
                        pbt[(kb, h)] = t_

                    def queue_kb(kb):
                        qlo, qhi, llo, lhi = _span(kb)
                        pieces = [(a, b) for (a, b) in
                                  [(llo, min(lhi, 512)), (max(llo, 512), lhi)] if a < b]

                        def part1():
                            t0 = spsum.tile([128, 1024], F32, tag="sc", name=f"s{kb}_0")
                            t1 = spsum.tile([128, 1024], F32, tag="sc", name=f"s{kb}_1")
                            for (a, b2) in pieces:
                                mm_score(t0, 0, kb, a, b2)
                                mm_score(t1, 1, kb, a, b2)
                            exp_mask(t0, kb, 0)
                            exp_mask(t1, kb, 1)

                        def part2():
                            t2 = spsum.tile([128, 1024], F32, tag="sc", name=f"s{kb}_2")
                            for (a, b2) in pieces:
                                mm_score(t2, 2, kb, a, b2)
                            exp_mask(t2, kb, 2)
                            state["kb_done"] += 1

                        band_q.append(part1)
                        band_q.append(part2)

                    def pump(n=1):
                        if PUMP_INTERLEAVE:
                            for _ in range(n):
                                if band_q:
                                    band_q.popleft()()

                    def pump_all():
                        while band_q:
                            band_q.popleft()()

                    def do_pv(qs):
                        q0 = QSB * qs
                        kbs = list(range(max(0, 4 * qs - 2), min(NKB, 4 * qs + 6)))
                        for h in range(3):
                            po = opsum.tile([96, 512], F32, tag="po")
                            nc.tensor.matmul(po[0:65, :], vGp[:, h, :],
                                             exp_sg[:, q0:q0 + 512],
                                             start=True, stop=False)
                            for i, kb in enumerate(kbs):
                                k0 = KB * kb
                                qlo, qhi, llo, lhi = _span(kb)
                                a, b2 = max(qlo, q0), min(qhi, q0 + QSB)
                                la = a - (k0 - 2 * KB)
                                nc.tensor.matmul(po[0:65, a - q0:b2 - q0],
                                                 v_nat[:, kb, h, :],
                                                 pbt[(kb, h)][:, la:la + (b2 - a)],
                                                 start=False, stop=(i == len(kbs) - 1))
                            ob = osbp.tile([65, 512], F32, tag="ob")
                            if h == 1:
                                nc.scalar.copy(ob[:], po[0:65, :])
                                nc.scalar.dma_start(out_d[h, :, q0:q0 + 512], ob[:])
                            else:
                                nc.vector.tensor_copy(ob[:], po[0:65, :])
                                if h == 0:
                                    nc.sync.dma_start(out_d[h, :, q0:q0 + 512], ob[:])
                                else:
                                    nc.gpsimd.dma_start(out_d[h, :, q0:q0 + 512],
                                                        ob[:])
                            pump()

                    def pump_pv(nt):
                        while state["qs_done"] < NQSB \
                                and 4 * state["qs_done"] + 5 <= state["kb_done"] - 1 \
                                and state["qs_done"] <= nt - 1:
                            do_pv(state["qs_done"])
                            state["qs_done"] += 1

                    def do_go(blks):
                        # one 195-col MM per block: out[g, (h,c)] includes the
                        # wanted diagonal blocks og[32h:32h+32, 65h:65h+65];
                        # off-diagonal products are computed but ignored.
                        # Beats 3 head-MMs of 65 cols (issue-overhead-bound).
                        gp = apsum.tile([96, 3, 65], F32, tag="pp",
                                        name=f"go{blks[0]}")
                        for i, t in enumerate(blks):
                            nc.tensor.matmul(gp[:], pb_gT[:, t, :],
                                             v_nat[:, t, 3:6, :],
                                             start=(i == 0),
                                             stop=(i == len(blks) - 1))
                        for h in range(3):
                            nc.vector.tensor_copy(og_acc[32 * h:32 * h + 32, :],
                                                  gp[32 * h:32 * h + 32, h, :])

                    nxt = {0: (hst0, hst80), 1: (hst_1, hst8_1)}
                    for nt in range(NNT):
                        c0 = 512 * nt
                        hst, hst8 = nxt.pop(nt)
                        # queue band blocks whose q/k spans are fully written
                        # (chunks <= nt-1); the rest after this chunk's m-chains
                        while state["kb_next"] <= min(4 * nt - 3, NKB - 1):
                            queue_kb(state["kb_next"])
                            state["kb_next"] += 1
                        # band work fills the PE while this chunk's hst8 DMA
                        # is still in flight
                        pump(2)
                        for c in range(5):
                            ps = apsum.tile([128, 512], F32, tag="pp")
                            for k2 in range(NK2):
                                nc.tensor.matmul(ps[:], w58_t[:, c, k2],
                                                 hst8[:, k2],
                                                 start=(k2 == 0), stop=(k2 == NK2 - 1),
                                                 perf_mode=DR)
                            if c == 0:
                                nc.vector.tensor_scalar(
                                    qZ[0][0:64, c0:c0 + 512], ps[0:64, :],
                                    DQ_Q, bias_t[0:64, 0:1], op0=OP_MUL, op1=OP_ADD)
                                nc.vector.tensor_scalar(
                                    qZ[1][64:128, c0:c0 + 512], ps[64:128, :],
                                    DQ_Q, bias_t[64:128, 0:1], op0=OP_MUL, op1=OP_ADD)
                            elif c == 1:
                                nc.vector.tensor_scalar(
                                    kT01[:, c0:c0 + 512], ps[:],
                                    DQ_K, bias_t[:, 1:2], op0=OP_MUL, op1=OP_ADD)
                            elif c == 2:
                                nc.vector.tensor_scalar(
                                    kgT01[:, c0:c0 + 512], ps[:],
                                    DQ_K, bias_t[:, 2:3], op0=OP_MUL, op1=OP_ADD)
                            elif c == 3:
                                nc.vector.tensor_scalar(
                                    qZ[2][0:64, c0:c0 + 512], ps[0:64, :],
                                    DQ_Q, bias_t[0:64, 3:4], op0=OP_MUL, op1=OP_ADD)
                                nc.vector.tensor_scalar(
                                    kgT2[64:128, c0:c0 + 512], ps[64:128, :],
                                    DQ_K, bias_t[64:128, 3:4], op0=OP_MUL, op1=OP_ADD)
                            else:
                                nc.vector.tensor_scalar(
                                    kT2[0:64, c0:c0 + 512], ps[0:64, :],
                                    DQ_K, bias_t[0:64, 4:5], op0=OP_MUL, op1=OP_ADD)
                                if nt == 0:
                                    nc.vector.tensor_scalar(
                                        qgp[64:128, 64:96], ps[64:128, 0:G],
                                        DQ_Q, bias_t[64:128, 4:5],
                                        op0=OP_MUL, op1=OP_ADD)
                            pump()
                        while state["kb_next"] <= min(4 * nt + 1, NKB - 1):
                            queue_kb(state["kb_next"])
                            state["kb_next"] += 1
                        if nt == 0:
                            psq = apsum.tile([128, 512], F32, tag="pp")
                            for kt in range(NKT):
                                nc.tensor.matmul(psq[:, 0:G], wqg_t[:, kt, :],
                                                 hst[:, kt, 0:G],
                                                 start=(kt == 0), stop=(kt == NKT - 1))
                            nc.vector.tensor_scalar_add(qgp[0:64, 0:32], psq[0:64, 0:G],
                                                        bias_t[0:64, 5:6])
                            nc.vector.tensor_scalar_add(qgp[64:128, 32:64],
                                                        psq[64:128, 0:G],
                                                        bias_t[64:128, 5:6])
                            # broadcast bvvg to all partitions via ones-matmul
                            psb = apsum.tile([128, 512], F32, tag="pp")
                            nc.tensor.matmul(psb[:, 0:384], ones_t[:, 0:128], bvvg_t[:],
                                             start=True, stop=True)
                            nc.vector.tensor_copy(bvvg_b[:], psb[:, 0:384])
                        for s4 in range(4):
                            sb = 4 * nt + s4
                            psv = apsum.tile([128, 512], F32, tag="pp")
                            for kt in range(NKT):
                                nc.tensor.matmul(psv[:, 0:384],
                                                 hst[:, kt, 128 * s4:128 * s4 + 128],
                                                 wvvg_t[:, kt, :],
                                                 start=(kt == 0), stop=(kt == NKT - 1))
                            nc.vector.tensor_add(
                                v_nat[:, sb, :, 0:64],
                                psv[:, 0:384].rearrange("p (h e) -> p h e", h=6),
                                bvvg_b[:, 0:384].rearrange("p (h e) -> p h e", h=6))
                            pump()
                        if nt == 0:
                            # per-head global v/ones at partitions 32h:32h+32
                            nc.vector.tensor_copy(vGp[0:32, 0, 0:64],
                                                  v_nat[0:32, 0, 0, 0:64])
                            nc.sync.dma_start(vGp[32:64, 1, 0:64],
                                              v_nat[0:32, 0, 1, 0:64])
                            nc.sync.dma_start(vGp[64:96, 2, 0:64],
                                              v_nat[0:32, 0, 2, 0:64])
                        pump_pv(nt)
                        pump(2)
                        # ---- (128,32)-col-tiled scores: sg + global-query ----
                        pssg = opsum.tile([96, 512], F32, tag="po", name=f"sg{nt}")
                        for h in range(3):
                            nc.tensor.matmul(pssg[32 * h:32 * h + 32, :],
                                             ktile(h)[:, 0:G],
                                             qZ[h][:, c0:c0 + 512])
                        nc.scalar.activation(exp_sg[0:96, c0:c0 + 512], pssg[:], AF.Exp)
                        # global-query scores computed TRANSPOSED (tokens on
                        # partitions): kg block is the stationary operand, so the
                        # exp'd probs land in pb_gT directly -- no PE transpose
                        gcb = apsum.tile([128, 4, 96], F32, tag="pp", name=f"gc{nt}")
                        for j in range(4):
                            tb = 128 * (4 * nt + j)
                            nc.tensor.matmul(gcb[:, j, 0:64],
                                             kgT01[:, tb:tb + 128], qgp[:, 0:64])
                            nc.tensor.matmul(gcb[:, j, 64:96],
                                             kgT2[:, tb:tb + 128], qgp[:, 64:96])
                        nc.scalar.activation(pb_gT[:, 4 * nt:4 * nt + 4, :], gcb[:],
                                             AF.Exp)
                        pump(2)
                        if nt + 2 < NNT:
                            # prefetch chunk nt+2's inputs so the next-next
                            # iteration's chains never wait on the DMA
                            h16 = hstp.tile([128, NKT, 512], BF, tag="h16")
                            h8 = hstp.tile([128, NK2, 2, 512], F8, tag="h8")
                            nc.sync.dma_start(h8[:], hsT8_d[:, nt + 2])
                            nc.sync.dma_start(h16[:], hsT_d[:, nt + 2, :, :])
                            nxt[nt + 2] = (h16, h8)

                    # tail: drain band queue, pv(6); the global-query PV chain
                    # runs before pv(7) so its copies/DMA overlap pv(7)'s matmuls
                    pump_all()
                    while state["qs_done"] <= 6:
                        do_pv(state["qs_done"])
                        state["qs_done"] += 1
                    while state["kb_next"] < NKB:
                        queue_kb(state["kb_next"])
                        state["kb_next"] += 1
                    pump_all()
                    do_go(list(range(NKB)))
                    do_pv(7)
                    nc.sync.dma_start(outg_d[0], og_acc[0:32, :])
                    nc.scalar.dma_start(outg_d[1], og_acc[32:64, :])
                    nc.gpsimd.dma_start(outg_d[2], og_acc[64:96, :])

    nc.compile()
    return nc


def _prep_inputs(inputs):
    hs = np.asarray(inputs["hidden_states"], dtype=np.float32)
    j = np.arange(KB)[None, :]
    p = np.arange(KB)[:, None]
    masks = np.stack([(j >= p), (j <= p)], axis=1).astype(bf16)  # [128, 2, 128]

    def wtiles(w):
        n = w.shape[1]
        return np.ascontiguousarray(w.reshape(NKT, 128, n).transpose(1, 0, 2)).astype(bf16)

    def fp8(x):
        return np.clip(x, -240.0, 240.0).astype(f8e4)

    maps = []
    for c in range(8):
        b, hg = c // 4, c % 4
        cols = slice(192 * hg, 192 * hg + 192)
        Wq = np.asarray(inputs["Wq"], np.float32)[:, cols] * SCALE
        bq = np.asarray(inputs["bq"], np.float32)[cols] * SCALE
        Wqg = np.asarray(inputs["Wqg"], np.float32)[:, cols] * SCALE
        bqg = np.asarray(inputs["bqg"], np.float32)[cols] * SCALE
        Wk = np.asarray(inputs["Wk"], np.float32)[:, cols]
        bk = np.asarray(inputs["bk"], np.float32)[cols]
        Wkg = np.asarray(inputs["Wkg"], np.float32)[:, cols]
        bkg = np.asarray(inputs["bkg"], np.float32)[cols]
        Wv = np.asarray(inputs["Wv"], np.float32)[:, cols]
        bv = np.asarray(inputs["bv"], np.float32)[cols]
        Wvg = np.asarray(inputs["Wvg"], np.float32)[:, cols]
        bvg = np.asarray(inputs["bvg"], np.float32)[cols]

        # 5 chains: q01, k01, kg01, [q2|kg2], [k2|qg2]; fp8-prescaled
        w5 = np.concatenate([
            Wq[:, 0:128] * WSQ,
            Wk[:, 0:128] * WSK,
            Wkg[:, 0:128] * WSK,
            np.concatenate([Wq[:, 128:192] * WSQ, Wkg[:, 128:192] * WSK], axis=1),
            np.concatenate([Wk[:, 128:192] * WSK, Wqg[:, 128:192] * WSQ], axis=1),
        ], axis=1)  # [768, 640]
        # channel c -> (k2 = c//256, i = (c//128)%2, p = c%128)
        w58 = np.ascontiguousarray(
            fp8(w5).reshape(NK2, 2, 128, 5, 128).transpose(2, 3, 0, 1, 4))

        bias_t = np.zeros((128, 8), np.float32)
        bias_t[:, 0] = bq[0:128]
        bias_t[:, 1] = bk[0:128]
        bias_t[:, 2] = bkg[0:128]
        bias_t[0:64, 3], bias_t[64:128, 3] = bq[128:192], bkg[128:192]
        bias_t[0:64, 4], bias_t[64:128, 4] = bk[128:192], bqg[128:192]
        bias_t[0:64, 5], bias_t[64:128, 5] = bqg[0:64], bqg[64:128]

        hsT = np.ascontiguousarray(
            hs[b].T.reshape(NKT, 128, NNT, 512).transpose(1, 2, 0, 3)).astype(bf16)
        hsT8 = np.ascontiguousarray(
            fp8(hs[b].T * XS).reshape(NK2, 2, 128, NNT, 512).transpose(2, 3, 0, 1, 4))

        maps.append({
            "hsT": hsT,
            "hsT8": hsT8,
            "w58": w58,
            "wqg01": wtiles(Wqg[:, 0:128]),
            "wvvg": wtiles(np.concatenate([Wv, Wvg], axis=1)),
            "bvvg": np.concatenate([bv, bvg])[None, :].astype(bf16),
            "bias_t": bias_t,
            "masks": masks,
        })
    return maps


def kernel(**inputs):
    g = int(np.asarray(inputs["num_global"]))
    assert g == G, f"kernel compiled for num_global=32, got {g}"
    if "nc" not in _cache:
        _cache["nc"] = _build()
    nc = _cache["nc"]
    in_maps = _prep_inputs(inputs)
    res = run_bass_kernel_spmd(nc, in_maps, list(range(8)))
    return assemble(res.results)


def assemble(results):
    out = np.zeros((B, S, D), np.float32)
    for c in range(8):
        b, hg = c // 4, c % 4
        o = results[c]["out"]          # [3, 65, S]
        og = results[c]["outg"]        # [3, G, 65]
        for h in range(3):
            col = 192 * hg + 64 * h
            out[b, :, col:col + 64] = (o[h, 0:64] / o[h, 64]).T
            out[b, 0:G, col:col + 64] = og[h, :, 0:64] / og[h, :, 64:65]
    return out
